# revision 1
# baseline (speedup 1.0000x reference)
"""Trainium2 Bass kernel for nn_Autoregression (16-state AR whitening log-prob).

Math: reference computes log_prob[b,k,t] = -0.5*(C*log(2pi) + logdet(Sigma_k)
+ es_k(t)^T Sigma_k^{-1} es_k(t)) with es = causal_conv(x, W, b).  Since
Sigma^{-1} = L^{-T} L^{-1} and es is affine in x, fold L^{-1} into the conv:
W2 = L^{-1} W, b2 = L^{-1} b, then mahalanobis = sum_c conv(x; W2, b2)^2.

Device layout (per core, T sharded 8 ways with an 8-sample left halo):
conv as matmuls over 128-t chunks producing PSUM [128 t, 512 (8 states x 64
ch)] x 2 halves; contraction packed as (c_in x 2 time-shifts)=128 rows per
step, 4 steps + a 65-row step for the j=8 tap whose ones-row carries the
bias.  ACT squares PSUM -> bf16 SBUF; DVE does the per-state segmented
reduce [128, 8, 64] -> [128, 8]; a small PE transpose flips [128 t, 16 k]
-> [16 k, 128 t]; DVE applies -0.5 and the per-state constant; DMA out.
"""

import os

import numpy as np
import ml_dtypes

import concourse.bass as bass
import concourse.bacc as bacc_mod
import concourse.mybir as mybir
import concourse.tile as tile
from concourse.bass_utils import run_bass_kernel_spmd
from concourse.tile_rust import add_dep_helper

K = 16          # states
C = 64          # channels
T = 65536       # time
AR = 8          # ar order (kernel size AR+1)
NCORES = 8
TLOC = T // NCORES          # 8192 outputs per core
TC = 128                    # outputs per chunk (matmul M)
WAVE = 16                   # chunks per wave (input tile granularity)
WCOLS = TC * WAVE           # 2048 outputs per wave
NW = TLOC // WCOLS          # waves per core
KP = K // 2
NSTEP = 5                   # contraction steps: 4 full + 1 (j=8 + bias row)
NH = 2                      # psum halves (states 0-7, 8-15)

MM_DT = mybir.dt.bfloat16   # conv matmul dtype
SQ_DT = mybir.dt.bfloat16   # squares dtype

_MM_NP = mybir.dt.np(MM_DT)

_CACHE: dict = {}


def _build_program():
    nc = bacc_mod.Bacc()
    f32 = mybir.dt.float32

    # xin rows 0-63: x slice (with halo); rows 64-127: same shifted left by 1
    # (host-duplicated so each wave's xd tile loads with a single DMA)
    xin = nc.declare_dram_parameter("xin", [128, TLOC + AR], MM_DT, isOutput=False)
    # weights as the matmul moving operand: [contraction, step, (half, state, ch)]
    wts = nc.declare_dram_parameter("wts", [128, NSTEP, 1024], MM_DT, isOutput=False)
    ident = nc.declare_dram_parameter("ident", [128, 128], mybir.dt.float32r, isOutput=False)
    biasc = nc.declare_dram_parameter("biasc", [K, 1], f32, isOutput=False)
    onesd = nc.declare_dram_parameter("onesd", [1, WCOLS], MM_DT, isOutput=False)
    out = nc.declare_dram_parameter("out", [K, TLOC], f32, isOutput=True)

    XDW = WCOLS + AR

    with tile.TileContext(nc) as tc:
        with (
            tc.tile_pool(name="singles", bufs=1) as singles,
            # one slot per wave: input DMAs never wait (no slot WAR/WAW)
            tc.tile_pool(name="xpool", bufs=NW) as xpool,
            tc.tile_pool(name="sqpool", bufs=12) as sqpool,
            tc.tile_pool(name="mpool", bufs=6) as mpool,
            tc.tile_pool(name="conv_ps", bufs=5, space="PSUM") as conv_ps,
            tc.tile_pool(name="mt_ps", bufs=2, space="PSUM") as mt_ps,
            tc.tile_pool(name="obs_ps", bufs=1, space="PSUM") as obs_ps,
        ):
            # Matmuls must never be the first PE instruction to observe more
            # than one producer semaphore (1-wait ISA slots; bacc's event-sem
            # legalization costs sequencer time).  pe_observe() emits a tiny
            # 2x2 "reader" matmul whose operands come from a single
            # producer's tile; ordering edges pin readers ahead of the next
            # real matmul.
            scratch = obs_ps.tile([2, 128], f32)
            scratch2 = singles.tile([2, 128], SQ_DT)
            nc.vector.memset(scratch2, 0.0)
            pending = []
            obs_after = [None]

            def pe_observe(col):
                i = nc.tensor.matmul(
                    scratch[0:2, 0:2], col, col, start=True, stop=True
                )
                if obs_after[0] is not None:
                    # not earlier than late in the previous wave, or the PE
                    # FIFO head-of-line blocks on a DMA that hasn't landed
                    add_dep_helper(i.ins, obs_after[0].ins, sync=False)
                pending.append(i)

            def _flush(i):
                while pending:
                    add_dep_helper(i.ins, pending.pop().ins, sync=False)
                return i

            def pe_matmul(*args, **kw):
                return _flush(nc.tensor.matmul(*args, **kw))

            # dep-free warmup matmuls: keep the PE busy through the initial
            # input DMAs so HAM un-throttles before real work (N=128 streams
            # so the activity monitor sees a busy array)
            for _ in range(35):
                nc.tensor.matmul(
                    scratch[0:2, 0:128],
                    scratch2[0:2, 0:2],
                    scratch2[0:2, 0:128],
                    start=True,
                    stop=True,
                )

            # DMA issue plan: sync HWDGE ring carries the critical path
            # (first xd half, per-step weights, second xd half);
            # prefetchables (identity, bias, xe, waves 1-3) go on the scalar
            # engine's separate ring.
            w_sb = singles.tile([128, NSTEP, 1024], MM_DT)
            ident_sb = singles.tile([128, 128], mybir.dt.float32r)
            bias_sb = singles.tile([K, 1], f32)
            out_sb = singles.tile([K, TLOC], f32)
            xds, xes = [], []
            sc_dmas = []
            sc_dmas.append(nc.scalar.dma_start(out=ident_sb, in_=ident[:, :]))
            sc_dmas.append(nc.scalar.dma_start(out=bias_sb, in_=biasc[:, :]))
            for w in range(NW):
                base = w * WCOLS
                # xd: rows 0-63 = xin shifts (j even), rows 64-127 = xin
                # shifted one further (j odd).  xe: rows 0-63 = xin shift 8,
                # row 64 = ones (bias row).
                xd = xpool.tile([128, XDW], MM_DT, name="xd")
                xe = xpool.tile([C + 1, WCOLS], MM_DT, name="xe")
                if w == 0:
                    nc.sync.dma_start(out=xd[:, 0:1036], in_=xin[:, 0:1036])
                    for s in range(NSTEP):
                        nc.sync.dma_start(
                            out=w_sb[:, s, :], in_=wts[:, s, :]
                        )
                    nc.sync.dma_start(out=xd[:, 1036:XDW], in_=xin[:, 1036:XDW])
                    sc_dmas.append(
                        nc.scalar.dma_start(
                            out=xe[0:C, :],
                            in_=xin[0:C, base + AR : base + AR + WCOLS],
                        )
                    )
                    sc_dmas.append(
                        nc.scalar.dma_start(out=xe[C : C + 1, :], in_=onesd[:, :])
                    )
                elif w == 1:
                    sc_dmas.append(
                        nc.scalar.dma_start(out=xd, in_=xin[:, base : base + XDW])
                    )
                    sc_dmas.append(
                        nc.scalar.dma_start(
                            out=xe[0:C, :],
                            in_=xin[0:C, base + AR : base + AR + WCOLS],
                        )
                    )
                    sc_dmas.append(
                        nc.scalar.dma_start(out=xe[C : C + 1, :], in_=onesd[:, :])
                    )
                xds.append(xd)
                xes.append(xe)

            def load_wave_inputs(w):
                # waves 2-3 load lazily (two waves ahead) so the prefetch
                # doesn't flood the DMA fabric while wave 0 computes
                base = w * WCOLS
                nc.scalar.dma_start(out=xds[w], in_=xin[:, base : base + XDW])
                nc.scalar.dma_start(
                    out=xes[w][0:C, :],
                    in_=xin[0:C, base + AR : base + AR + WCOLS],
                )
                nc.scalar.dma_start(out=xes[w][C : C + 1, :], in_=onesd[:, :])
            # DVE observer for the bias DMA (TS struct fits one wait)
            dve_scratch = singles.tile([K, 1], f32)
            nc.vector.tensor_copy(dve_scratch, bias_sb)

            first_sq = [True]

            def chunk_tail(w, off, psh):
                base = w * WCOLS
                m_sb = mpool.tile([128, K], mybir.dt.float32r, name="m_sb")
                for h in range(NH):
                    sq = sqpool.tile([128, 512], SQ_DT, name="sq", tag="sq")
                    sq_i = nc.scalar.activation(
                        sq, psh[h], mybir.ActivationFunctionType.Square
                    )
                    if first_sq[0]:
                        # the Act sequencer must issue every prefetch DMA
                        # before its first square, else a square that
                        # transitively gates one of those DMAs deadlocks
                        while sc_dmas:
                            add_dep_helper(sq_i.ins, sc_dmas.pop().ins, sync=False)
                        first_sq[0] = False
                    with nc.allow_low_precision(
                        reason="float32r shares float32 bits; r-mode only "
                        "affects the PE multiply path"
                    ):
                        nc.vector.tensor_reduce(
                            out=m_sb[:, 8 * h : 8 * h + 8],
                            in_=sq.rearrange("p (g c) -> p g c", g=8),
                            axis=mybir.AxisListType.X,
                            op=mybir.AluOpType.add,
                        )
                mt = mt_ps.tile([K, TC], mybir.dt.float32r, name="mt")
                _flush(nc.tensor.transpose(mt, m_sb, ident_sb))
                nc.vector.tensor_scalar(
                    out=out_sb[:, base + off : base + off + TC],
                    in0=mt[0:K, :],
                    scalar1=-0.5,
                    scalar2=bias_sb,
                    op0=mybir.AluOpType.mult,
                    op1=mybir.AluOpType.add,
                )

            def conv_lhsT(xd, xe, off, s):
                if s < 4:
                    return xd[:, off + 2 * s : off + 2 * s + TC]
                return xe[0 : C + 1, off : off + TC]

            def conv_rhs(s, h):
                if s < 4:
                    return w_sb[:, s, 512 * h : 512 * h + 512]
                return w_sb[0 : C + 1, s, 512 * h : 512 * h + 512]

            for w in range(NW):
                base = w * WCOLS
                xd = xds[w]
                xe = xes[w]
                if w + 2 < NW:
                    load_wave_inputs(w + 2)
                if w == 0:
                    # first four chunks pairwise s-major: the weight steps
                    # arrive one-by-one on the sync ring, so advance both
                    # chunks per step instead of stalling per chunk
                    for pair in ((0, 1), (2, 3)):
                        pshp = {
                            (c, h): conv_ps.tile(
                                [128, 512], f32, name=f"ps{c}{h}", tag="ps"
                            )
                            for c in pair
                            for h in range(NH)
                        }
                        for s in range(NSTEP):
                            if pair[0] == 0 and s == 0:
                                pe_observe(ident_sb[:, 0:2])
                                pe_observe(xd[:, 0:2])
                            if pair[0] == 0 and s == 4:
                                pe_observe(xe[0:C, 0:2])
                                pe_observe(xe[C : C + 1, 0:2])
                            for c in pair:
                                lhsT = conv_lhsT(xd, xe, c * TC, s)
                                for h in range(NH):
                                    pe_matmul(
                                        pshp[c, h],
                                        lhsT,
                                        conv_rhs(s, h),
                                        start=(s == 0),
                                        stop=(s == 4),
                                    )
                        for c in pair:
                            chunk_tail(w, c * TC, [pshp[c, h] for h in range(NH)])
                    start_tcl = 4
                else:
                    start_tcl = 0
                for tcl in range(start_tcl, WAVE):
                    off = tcl * TC
                    psh = [
                        conv_ps.tile([128, 512], f32, name=f"ps{h}", tag="ps")
                        for h in range(NH)
                    ]
                    for s in range(NSTEP):
                        if w > 0 and tcl == 0 and s == 0:
                            pe_observe(xd[:, 0:2])
                        if w > 0 and tcl == 0 and s == 4:
                            # lazily: s0-s3 must not stall on the xe loads
                            pe_observe(xe[0:C, 0:2])
                            pe_observe(xe[C : C + 1, 0:2])
                        lhsT = conv_lhsT(xd, xe, off, s)
                        for h in range(NH):
                            mm_i = pe_matmul(
                                psh[h],
                                lhsT,
                                conv_rhs(s, h),
                                start=(s == 0),
                                stop=(s == 4),
                            )
                    if tcl == WAVE - 2:
                        obs_after[0] = mm_i
                    chunk_tail(w, off, psh)
                if w < NW - 1:
                    nc.sync.dma_start(
                        out=out[:, base : base + WCOLS],
                        in_=out_sb[:, base : base + WCOLS],
                    )
                else:
                    # last wave: quarter DMAs so the final store is tiny
                    for q in range(4):
                        uq = base + q * (WCOLS // 4)
                        nc.sync.dma_start(
                            out=out[:, uq : uq + WCOLS // 4],
                            in_=out_sb[:, uq : uq + WCOLS // 4],
                        )
    nc.compile()
    return nc


def _prep_host(W, b, Sigma):
    """Fold L^{-1} into conv weights; pack moving-operand tiles, constants."""
    W64 = W.astype(np.float64)
    b64 = b.astype(np.float64)
    S64 = Sigma.astype(np.float64)
    L = np.linalg.cholesky(S64)
    Li = np.linalg.inv(L)                       # [K, C, C] lower-triangular inv
    logdet = 2.0 * np.sum(np.log(np.diagonal(L, axis1=1, axis2=2)), axis=1)
    W2 = np.einsum("kdc,kcij->kdij", Li, W64)   # [K, C(d), C(ci), 9]
    b2 = np.einsum("kdc,kc->kd", Li, b64)       # [K, C]

    # moving operand: w_np[r, s, 512*(k//8) + 64*(k%8) + d]
    #   s<4:  r = ci + 64*joff -> W2[k, d, ci, 2s+joff]
    #   s==4: r<64 -> W2[k, d, r, 8]; r==64 -> b2[k, d]; else 0
    w_np = np.zeros((128, NSTEP, 1024), np.float32)
    for s in range(4):
        # [ci + 64*joff, (k, d)]
        sub = W2[:, :, :, 2 * s : 2 * s + 2]        # [k, d, ci, joff]
        w_np[:, s, :] = np.transpose(sub, (3, 2, 0, 1)).reshape(128, 1024)
    w_np[0:C, 4, :] = np.transpose(W2[:, :, :, 8], (2, 0, 1)).reshape(C, 1024)
    w_np[C, 4, :] = b2.reshape(1024)

    const = C * np.log(2.0 * np.pi) + logdet
    bias_np = (-0.5 * const).astype(np.float32).reshape(K, 1)
    return w_np, bias_np


def _run(x, W, b, Sigma, trace=False):
    x = np.asarray(x, np.float32)
    W = np.asarray(W, np.float32)
    b = np.asarray(b, np.float32)
    Sigma = np.asarray(Sigma, np.float32)
    if "nc" not in _CACHE:
        _CACHE["nc"] = _build_program()
    nc = _CACHE["nc"]
    w_np, bias_np = _prep_host(W, b, Sigma)

    # left causal pad (AR) plus one right pad col so the shifted copy of the
    # last core's slice stays in bounds
    xpad = np.pad(np.asarray(x, np.float32)[0], ((0, 0), (AR, 1)))  # [C, T+9]
    in_maps = []
    for i in range(NCORES):
        lo = xpad[:, TLOC * i : TLOC * i + TLOC + AR]
        hi = xpad[:, TLOC * i + 1 : TLOC * i + TLOC + AR + 1]
        in_maps.append(
            {
                "xin": np.ascontiguousarray(
                    np.concatenate([lo, hi], axis=0).astype(_MM_NP)
                ),
                "wts": w_np.astype(_MM_NP),
                "ident": np.eye(128, dtype=np.float32),
                "biasc": bias_np,
                "onesd": np.ones((1, WCOLS), _MM_NP),
            }
        )
    res = run_bass_kernel_spmd(
        nc, in_maps, core_ids=list(range(NCORES)), trace=trace
    )
    outs = [res.results[i]["out"] for i in range(NCORES)]
    full = np.concatenate(outs, axis=1)[None]   # [1, K, T]
    return full.astype(np.float32), res


def kernel(x, W, b, Sigma):
    out, _ = _run(x, W, b, Sigma, trace=bool(int(os.environ.get("BASS_TRACE", "0"))))
    return out



# revision 3
# speedup vs baseline: 1.7804x; 1.7804x over previous
"""Trainium2 Bass kernel for nn_Autoregression (16-state AR whitening log-prob).

Math: log_prob[b,k,t] = -0.5*(C*log(2pi) + logdet(Sigma_k)
+ es_k(t)^T Sigma_k^{-1} es_k(t)) with es = causal_conv(x, W, b).  Fold
L^{-1} into the conv (W2 = L^{-1} W, b2 = L^{-1} b) so mahalanobis =
sum_d conv(x; W2, b2)^2.

Device layout (per core, T sharded 8 ways with an 8-sample causal halo):
conv runs in fp8e4 DoubleRow matmuls (256-deep contraction per
instruction, 2 k-tiles along the free axis).  Contraction of 64 ci x 9
taps + bias = 577 rows packs as 2 full DoubleRows from a shift-duplicated
x tile (taps 0-3 / 4-7 via kt-plane shifts of 2) plus one 66-row
DoubleRow (tap 8 + ones-bias row).  Per 128-t chunk: 6 matmuls into a
2-bank PSUM tile [128 t, 1024 (2 halves x 8 states x 64 ch)].  ACT
squares the whole tile in one op (scale folds the fp8 weight scale and
the -0.5) -> fp16 SBUF; DVE folds the d-dim in two steps (a 2x-mode
tensor_tensor add of d-halves, then a 1x segmented reduce) -> [128,16]
fp16.  Every 8 chunks a fp16 PE transpose flips [128 t, 128 (c,k)] ->
[128, 128 t]; DVE applies -1 and the per-state constant (2x mode);
fp16 k-major stores, host converts to f32.
"""

import math
import os

import numpy as np
import ml_dtypes

import concourse.bass as bass
import concourse.bacc as bacc_mod
import concourse.mybir as mybir
import concourse.tile as tile
from concourse.bass_utils import run_bass_kernel_spmd
from concourse.tile_rust import add_dep_helper

K = 16          # states
C = 64          # channels
T = 65536       # time
AR = 8          # ar order (kernel size AR+1)
NCORES = 8
TLOC = T // NCORES          # 8192 outputs per core
TC = 128                    # outputs per chunk (matmul M)
WAVE = 16                   # chunks per wave (input tile granularity)
WCOLS = TC * WAVE           # 2048 outputs per wave
NW = TLOC // WCOLS          # waves per core
GRP = 8                     # chunks per transpose/store group
LAG = 3                     # chunks between a group's last reduce and its transpose
XW = WCOLS + 16             # xd3 plane width (halo, 16B-aligned kt stride)
XEW = WCOLS                 # xe3 plane width
SW = 64.0                   # fp8 weight scale

F8 = mybir.dt.float8e4
F16 = mybir.dt.float16
NP8 = mybir.dt.np(F8)
NP16 = mybir.dt.np(F16)
DR = mybir.MatmulPerfMode.DoubleRow

_CACHE: dict = {}


def _build_program():
    nc = bacc_mod.Bacc()
    f32 = mybir.dt.float32

    # x, shift-duplicated: partition p=(joff,ci), plane kt -> time shift
    # joff + 2*kt; per wave 128 rows of [2, XW]
    xdram = nc.declare_dram_parameter("xdram", [NW * 128, 2, XW], F8, isOutput=False)
    # tail operand: rows 0-63 = x shifted by 8 (tap 8), row 64 = ones
    # (bias row), row 65 = pad; kt1 = copy of kt0 (zero-weighted)
    xedram = nc.declare_dram_parameter("xedram", [NW * 66, 2, XEW], F8, isOutput=False)
    # conv weights, moving operand: [row=(joff,ci), kt, dr*1024 + h*512 + k8*64 + d]
    wdram = nc.declare_dram_parameter("wdram", [128, 2, 2048], F8, isOutput=False)
    # tail weights: [row, kt, h*512 + k8*64 + d]; row 64 = bias, kt1 = 0
    wtdram = nc.declare_dram_parameter("wtdram", [66, 2, 1024], F8, isOutput=False)
    identd = nc.declare_dram_parameter("identd", [128, 128], F16, isOutput=False)
    # -0.5 * (C*log(2pi) + logdet_k) per transposed partition p -> k = p%16
    biasd = nc.declare_dram_parameter("biasd", [128, 1], f32, isOutput=False)
    out = nc.declare_dram_parameter("out", [K, TLOC], F16, isOutput=True)

    NG = NW * WAVE // GRP       # groups per core
    ACT_SCALE = 1.0 / (SW * math.sqrt(2.0))

    with tile.TileContext(nc) as tc:
        with (
            tc.tile_pool(name="singles", bufs=1) as singles,
            # one slot per wave: input DMAs never wait (no slot WAR/WAW)
            tc.tile_pool(name="xpool", bufs=NW) as xpool,
            tc.tile_pool(name="sqpool", bufs=6) as sqpool,
            tc.tile_pool(name="t1pool", bufs=4) as t1pool,
            tc.tile_pool(name="mgpool", bufs=2) as mgpool,
            tc.tile_pool(name="owpool", bufs=2) as owpool,
            tc.tile_pool(name="conv_ps", bufs=3, space="PSUM") as conv_ps,
            tc.tile_pool(name="mt_ps", bufs=1, space="PSUM") as mt_ps,
        ):
            # Matmuls must never be the first PE instruction to observe more
            # than one producer semaphore (1-wait ISA slots; bacc's event-sem
            # legalization costs sequencer time).  pe_observe() emits a tiny
            # 2x2 "reader" matmul whose operands come from a single
            # producer's tile; ordering edges pin readers ahead of the next
            # real matmul.
            scratch = mt_ps.tile([2, 128], f32)
            scratch2 = singles.tile([2, 128], mybir.dt.bfloat16)
            nc.vector.memset(scratch2, 0.0)
            pending = []
            obs_after = [None]

            def pe_observe(col):
                i = nc.tensor.matmul(
                    scratch[0:2, 0:2], col, col, start=True, stop=True
                )
                if obs_after[0] is not None:
                    # not earlier than late in the previous wave, or the PE
                    # FIFO head-of-line blocks on a DMA that hasn't landed
                    add_dep_helper(i.ins, obs_after[0].ins, sync=False)
                pending.append(i)

            def _flush(i):
                while pending:
                    add_dep_helper(i.ins, pending.pop().ins, sync=False)
                return i

            def pe_matmul(*args, **kw):
                return _flush(nc.tensor.matmul(*args, **kw))

            # dep-free warmup matmuls: keep the PE busy through the initial
            # input DMAs so HAM un-throttles before real work
            for _ in range(35):
                nc.tensor.matmul(
                    scratch[0:2, 0:128],
                    scratch2[0:2, 0:2],
                    scratch2[0:2, 0:128],
                    start=True,
                    stop=True,
                )

            w_sb = singles.tile([128, 2, 2048], F8)
            wt_sb = singles.tile([66, 2, 1024], F8)
            ident_sb = singles.tile([128, 128], F16)
            bias_sb = singles.tile([128, 1], f32)

            xds, xes = [], []
            for w in range(NW):
                xds.append(xpool.tile([128, 2, XW], F8, name="xd3"))
                xes.append(xpool.tile([66, 2, XEW], F8, name="xe3"))

            # wave 0 on the sync HWDGE ring, ordered so chunk 0's six
            # matmuls can start as early as possible
            nc.sync.dma_start(out=xds[0][:, :, 0:272], in_=xdram[0:128, :, 0:272])
            for blk in range(4):
                nc.sync.dma_start(
                    out=w_sb[:, :, 512 * blk : 512 * blk + 512],
                    in_=wdram[:, :, 512 * blk : 512 * blk + 512],
                )
            nc.sync.dma_start(out=wt_sb, in_=wtdram[:, :, :])
            nc.sync.dma_start(out=xes[0][:, :, 0:272], in_=xedram[0:66, :, 0:272])
            nc.sync.dma_start(out=xds[0][:, :, 272:1040], in_=xdram[0:128, :, 272:1040])
            nc.sync.dma_start(out=xes[0][:, :, 272:XEW], in_=xedram[0:66, :, 272:XEW])
            nc.sync.dma_start(out=xds[0][:, :, 1040:XW], in_=xdram[0:128, :, 1040:XW])
            nc.sync.dma_start(out=ident_sb, in_=identd[:, :])
            nc.sync.dma_start(out=bias_sb, in_=biasd[:, :])

            def load_wave_inputs(w):
                nc.sync.dma_start(
                    out=xds[w], in_=xdram[128 * w : 128 * w + 128, :, :]
                )
                nc.sync.dma_start(
                    out=xes[w], in_=xedram[66 * w : 66 * w + 66, :, :]
                )

            load_wave_inputs(1)

            mg_tiles = [None] * NG

            def emit_group_tail(g):
                mt = mt_ps.tile([128, 128], F16, name="mt")
                _flush(nc.tensor.transpose(mt, mg_tiles[g], ident_sb))
                ow = owpool.tile([128, 128], F16, name="ow")
                with nc.allow_low_precision(reason="fp16 output"):
                    nc.vector.tensor_scalar(
                        out=ow,
                        in0=mt,
                        scalar1=-1.0,
                        scalar2=bias_sb,
                        op0=mybir.AluOpType.mult,
                        op1=mybir.AluOpType.add,
                    )
                seg = out[:, g * GRP * TC : (g + 1) * GRP * TC]
                nc.sync.dma_start(
                    out=seg.rearrange("k (c tc) -> c k tc", c=GRP),
                    in_=ow,
                )

            for w in range(NW):
                xd3 = xds[w]
                xe3 = xes[w]
                if w + 2 < NW:
                    load_wave_inputs(w + 2)
                for tcl in range(WAVE):
                    c = w * WAVE + tcl
                    g = c // GRP
                    cl = c % GRP
                    off = tcl * TC
                    if tcl == 0:
                        pe_observe(xd3[0:2, 0, 0:2])
                        if w == 0:
                            pe_observe(ident_sb[0:2, 0:2])
                    if tcl == 1:
                        pe_observe(xe3[0:2, 0, 0:2])

                    ps = conv_ps.tile([128, 1024], f32, name="ps", tag="ps")
                    for dr in range(2):
                        lhsT = xd3[:, :, off + 4 * dr : off + 4 * dr + TC]
                        for h in range(2):
                            pe_matmul(
                                ps[:, 512 * h : 512 * h + 512],
                                lhsT,
                                w_sb[:, :, dr * 1024 + h * 512 : dr * 1024 + h * 512 + 512],
                                start=(dr == 0),
                                stop=False,
                                perf_mode=DR,
                            )
                    lhsT3 = xe3[:, :, off : off + TC]
                    for h in range(2):
                        mm_i = pe_matmul(
                            ps[:, 512 * h : 512 * h + 512],
                            lhsT3,
                            wt_sb[:, :, 512 * h : 512 * h + 512],
                            start=False,
                            stop=True,
                            perf_mode=DR,
                        )
                    if tcl == WAVE - 2:
                        obs_after[0] = mm_i

                    sq = sqpool.tile([128, 1024], F16, name="sq", tag="sq")
                    nc.scalar.activation(
                        sq, ps[:, 0:1024],
                        mybir.ActivationFunctionType.Square,
                        scale=ACT_SCALE,
                    )
                    if cl == 0:
                        mg_tiles[g] = mgpool.tile([128, GRP * 16], F16, name="mg")
                    sq3 = sq.rearrange("p (g c) -> p g c", g=16)
                    t1 = t1pool.tile([128, 16, 32], F16, name="t1", tag="t1")
                    with nc.allow_low_precision(reason="fp16 mahalanobis"):
                        # d-halves pairwise add runs in DVE 2x mode; the
                        # remaining 32-wide segmented reduce runs 1x
                        nc.vector.tensor_tensor(
                            out=t1,
                            in0=sq3[:, :, 0:32],
                            in1=sq3[:, :, 32:64],
                            op=mybir.AluOpType.add,
                        )
                        nc.vector.tensor_reduce(
                            out=mg_tiles[g][:, 16 * cl : 16 * cl + 16],
                            in_=t1,
                            axis=mybir.AxisListType.X,
                            op=mybir.AluOpType.add,
                        )

                    # emit group go's transpose/affine/store LAG chunks after
                    # its last reduce so the in-order PE never head-of-line
                    # blocks on the DVE
                    if c >= (GRP - 1) + LAG and (c - (GRP - 1) - LAG) % GRP == 0:
                        emit_group_tail((c - (GRP - 1) - LAG) // GRP)

            first_left = (NG * GRP - 1 - (GRP - 1) - LAG) // GRP + 1
            for go in range(max(first_left, 0), NG):
                emit_group_tail(go)
    nc.compile()
    return nc


def _prep_host(W, b, Sigma):
    """Fold L^{-1} into conv weights; quantize and pack fp8 operands."""
    W64 = W.astype(np.float64)
    b64 = b.astype(np.float64)
    S64 = Sigma.astype(np.float64)
    L = np.linalg.cholesky(S64)
    Li = np.linalg.inv(L)
    logdet = 2.0 * np.sum(np.log(np.diagonal(L, axis1=1, axis2=2)), axis=1)
    W2 = np.einsum("kdc,kcij->kdij", Li, W64)   # [K, C(d), C(ci), 9]
    b2 = np.einsum("kdc,kc->kd", Li, b64)       # [K, C]

    W2s = (SW * W2).astype(np.float32)          # [kg, d, ci, j]
    w_main = np.zeros((128, 2, 2048), np.float32)
    for joff in range(2):
        for kt in range(2):
            for dr in range(2):
                j = 4 * dr + joff + 2 * kt
                # [ci, (h k8) d] = W2s[kg, d, ci, j]
                blk = np.transpose(W2s[:, :, :, j], (2, 0, 1)).reshape(C, 1024)
                w_main[64 * joff : 64 * joff + 64, kt,
                       1024 * dr : 1024 * dr + 1024] = blk
    w_tail = np.zeros((66, 2, 1024), np.float32)
    w_tail[0:C, 0, :] = np.transpose(W2s[:, :, :, 8], (2, 0, 1)).reshape(C, 1024)
    w_tail[C, 0, :] = (SW * b2).astype(np.float32).reshape(1024)

    const = C * np.log(2.0 * np.pi) + logdet    # [K]
    bias_np = np.tile((-0.5 * const).astype(np.float32), 8).reshape(128, 1)
    return w_main.astype(NP8), w_tail.astype(NP8), bias_np


def _prep_x(x):
    """Quantize x once, build per-core shift-duplicated fp8 operands."""
    xq = np.asarray(x, np.float32)[0].astype(NP8)            # [C, T]
    xpad = np.zeros((C, AR + T + 24), NP8)
    xpad[:, AR : AR + T] = xq
    xd_all, xe_all = [], []
    ones_row = np.ones((1, XEW), NP8)
    zero_row = np.zeros((1, XEW), NP8)
    for i in range(NCORES):
        xd = np.empty((NW * 128, 2, XW), NP8)
        xe = np.empty((NW * 66, 2, XEW), NP8)
        for w in range(NW):
            base = i * TLOC + w * WCOLS
            for joff in range(2):
                for kt in range(2):
                    s = base + joff + 2 * kt
                    xd[128 * w + 64 * joff : 128 * w + 64 * joff + 64, kt, :] = \
                        xpad[:, s : s + XW]
            tail = xpad[:, base + 8 : base + 8 + XEW]
            for kt in range(2):
                xe[66 * w : 66 * w + 64, kt, :] = tail
                xe[66 * w + 64, kt, :] = ones_row
                xe[66 * w + 65, kt, :] = zero_row
        xd_all.append(xd)
        xe_all.append(xe)
    return xd_all, xe_all


def _run(x, W, b, Sigma, trace=False):
    if "nc" not in _CACHE:
        _CACHE["nc"] = _build_program()
    nc = _CACHE["nc"]
    w_main, w_tail, bias_np = _prep_host(
        np.asarray(W, np.float32), np.asarray(b, np.float32),
        np.asarray(Sigma, np.float32))
    xd_all, xe_all = _prep_x(np.asarray(x, np.float32))
    ident = np.eye(128, dtype=NP16)

    in_maps = []
    for i in range(NCORES):
        in_maps.append(
            {
                "xdram": xd_all[i],
                "xedram": xe_all[i],
                "wdram": w_main,
                "wtdram": w_tail,
                "identd": ident,
                "biasd": bias_np,
            }
        )
    res = run_bass_kernel_spmd(
        nc, in_maps, core_ids=list(range(NCORES)), trace=trace
    )
    outs = [res.results[i]["out"].astype(np.float32) for i in range(NCORES)]
    full = np.concatenate(outs, axis=1)[None]   # [1, K, T]
    return full, res


def kernel(x, W, b, Sigma):
    out, _ = _run(x, W, b, Sigma, trace=bool(int(os.environ.get("BASS_TRACE", "0"))))
    return out


# revision 4
# speedup vs baseline: 1.8058x; 1.0143x over previous
"""Trainium2 Bass kernel for nn_Autoregression (16-state AR whitening log-prob).

Math: log_prob[b,k,t] = -0.5*(C*log(2pi) + logdet(Sigma_k)
+ es_k(t)^T Sigma_k^{-1} es_k(t)) with es = causal_conv(x, W, b).  Fold
L^{-1} into the conv (W2 = L^{-1} W, b2 = L^{-1} b) so mahalanobis =
sum_d conv(x; W2, b2)^2.

Device layout (per core, T sharded 8 ways with an 8-sample causal halo):
conv runs in fp8e4 DoubleRow matmuls (256-deep contraction per
instruction, 2 k-tiles along the free axis).  Contraction of 64 ci x 9
taps + bias = 577 rows packs as 2 full DoubleRows from a shift-duplicated
x tile (taps 0-3 / 4-7 via kt-plane shifts of 2) plus one 66-row
DoubleRow (tap 8 + ones-bias row).  Per 128-t chunk: 6 matmuls into a
2-bank PSUM tile [128 t, 1024 (2 halves x 8 states x 64 ch)].  ACT
squares the whole tile in one op (scale folds the fp8 weight scale and
the -0.5) -> fp16 SBUF; DVE folds the d-dim in two steps (a 2x-mode
tensor_tensor add of d-halves, then a 1x segmented reduce) -> [128,16]
fp16.  Every 8 chunks a fp16 PE transpose flips [128 t, 128 (c,k)] ->
[128, 128 t]; DVE applies -1 and the per-state constant (2x mode);
fp16 k-major stores, host converts to f32.
"""

import math
import os

import numpy as np
import ml_dtypes

import concourse.bass as bass
import concourse.bacc as bacc_mod
import concourse.mybir as mybir
import concourse.tile as tile
from concourse.bass_utils import run_bass_kernel_spmd
from concourse.tile_rust import add_dep_helper

K = 16          # states
C = 64          # channels
T = 65536       # time
AR = 8          # ar order (kernel size AR+1)
NCORES = 8
TLOC = T // NCORES          # 8192 outputs per core
TC = 128                    # outputs per chunk (matmul M)
WAVE = 16                   # chunks per wave (input tile granularity)
WCOLS = TC * WAVE           # 2048 outputs per wave
NW = TLOC // WCOLS          # waves per core
GRP = 8                     # chunks per transpose/store group
LAG = 3                     # chunks between a group's last reduce and its transpose
XW = WCOLS + 32             # xd3 plane width (halo + shift-copy margin, 16B-aligned)
XEW = WCOLS                 # xe3 plane width
SW = 64.0                   # fp8 weight scale

F8 = mybir.dt.float8e4
F16 = mybir.dt.float16
NP8 = mybir.dt.np(F8)
NP16 = mybir.dt.np(F16)
DR = mybir.MatmulPerfMode.DoubleRow

_CACHE: dict = {}


def _build_program():
    nc = bacc_mod.Bacc()
    f32 = mybir.dt.float32

    # x, shift-duplicated: partition p=(joff,ci), plane kt -> time shift
    # joff + 2*kt; per wave 128 rows of [2, XW]
    xdram = nc.declare_dram_parameter("xdram", [NW * 128, XW], F8, isOutput=False)
    # tail operand: rows 0-63 = x shifted by 8 (tap 8), row 64 = ones
    # (bias row), row 65 = pad; kt1 = copy of kt0 (zero-weighted)
    xedram = nc.declare_dram_parameter("xedram", [NW * 66, 2, XEW], F8, isOutput=False)
    # conv weights, moving operand: [row=(joff,ci), kt, dr*1024 + h*512 + k8*64 + d]
    wdram = nc.declare_dram_parameter("wdram", [128, 2, 2048], F8, isOutput=False)
    # tail weights: [row, kt, h*512 + k8*64 + d]; row 64 = bias, kt1 = 0
    wtdram = nc.declare_dram_parameter("wtdram", [66, 2, 1024], F8, isOutput=False)
    identd = nc.declare_dram_parameter("identd", [128, 128], F16, isOutput=False)
    # -0.5 * (C*log(2pi) + logdet_k) per transposed partition p -> k = p%16
    biasd = nc.declare_dram_parameter("biasd", [128, 1], f32, isOutput=False)
    out = nc.declare_dram_parameter("out", [K, TLOC], F16, isOutput=True)

    NG = NW * WAVE // GRP       # groups per core
    ACT_SCALE = 1.0 / (SW * math.sqrt(2.0))

    with tile.TileContext(nc) as tc:
        with (
            tc.tile_pool(name="singles", bufs=1) as singles,
            # one slot per wave: input DMAs never wait (no slot WAR/WAW)
            tc.tile_pool(name="xpool", bufs=NW) as xpool,
            tc.tile_pool(name="sqpool", bufs=6) as sqpool,
            tc.tile_pool(name="t1pool", bufs=4) as t1pool,
            tc.tile_pool(name="mgpool", bufs=2) as mgpool,
            tc.tile_pool(name="owpool", bufs=2) as owpool,
            tc.tile_pool(name="conv_ps", bufs=3, space="PSUM") as conv_ps,
            tc.tile_pool(name="mt_ps", bufs=1, space="PSUM") as mt_ps,
        ):
            # Matmuls must never be the first PE instruction to observe more
            # than one producer semaphore (1-wait ISA slots; bacc's event-sem
            # legalization costs sequencer time).  pe_observe() emits a tiny
            # 2x2 "reader" matmul whose operands come from a single
            # producer's tile; ordering edges pin readers ahead of the next
            # real matmul.
            scratch = mt_ps.tile([2, 128], f32)
            scratch2 = singles.tile([2, 128], mybir.dt.bfloat16)
            nc.vector.memset(scratch2, 0.0)
            pending = []
            obs_after = [None]

            def pe_observe(col):
                i = nc.tensor.matmul(
                    scratch[0:2, 0:2], col, col, start=True, stop=True
                )
                if obs_after[0] is not None:
                    # not earlier than late in the previous wave, or the PE
                    # FIFO head-of-line blocks on a DMA that hasn't landed
                    add_dep_helper(i.ins, obs_after[0].ins, sync=False)
                pending.append(i)

            def _flush(i):
                while pending:
                    add_dep_helper(i.ins, pending.pop().ins, sync=False)
                return i

            def pe_matmul(*args, **kw):
                return _flush(nc.tensor.matmul(*args, **kw))

            # dep-free warmup matmuls: keep the PE busy through the initial
            # input DMAs so HAM un-throttles before real work
            for _ in range(35):
                nc.tensor.matmul(
                    scratch[0:2, 0:128],
                    scratch2[0:2, 0:2],
                    scratch2[0:2, 0:128],
                    start=True,
                    stop=True,
                )

            w_sb = singles.tile([128, 2, 2048], F8)
            wt_sb = singles.tile([66, 2, 1024], F8)
            ident_sb = singles.tile([128, 128], F16)
            bias_sb = singles.tile([128, 1], f32)

            xds, xes = [], []
            for w in range(NW):
                xds.append(xpool.tile([128, 2, XW], F8, name="xd3"))
                xes.append(xpool.tile([66, 2, XEW], F8, name="xe3"))

            # All input loads ride the gpsimd SWDGE ring: software DGE fans
            # descriptors across all 16 DMA engines (~7x the bandwidth of a
            # single HWDGE ring).  Only kt-plane 0 of the x operand comes
            # from DRAM; kt1 (= kt0 shifted 2 cols) is synthesized by a DVE
            # uint16 shift-copy.
            U16 = mybir.dt.uint16

            def load_wave_inputs(w):
                nc.gpsimd.dma_start(
                    out=xds[w][:, 0, :], in_=xdram[128 * w : 128 * w + 128, :]
                )
                nc.gpsimd.dma_start(
                    out=xes[w], in_=xedram[66 * w : 66 * w + 66, :, :]
                )

            def dup_wave_kt1(w):
                nc.vector.tensor_copy(
                    xds[w][:, 1, 0 : XW - 16].bitcast(U16),
                    xds[w][:, 0, 2 : XW - 14].bitcast(U16),
                )

            load_wave_inputs(0)
            nc.gpsimd.dma_start(
                out=w_sb.rearrange("p a b -> p (a b)"),
                in_=wdram.rearrange("p a b -> p (a b)"),
            )
            nc.gpsimd.dma_start(out=wt_sb, in_=wtdram[:, :, :])
            nc.gpsimd.dma_start(out=ident_sb, in_=identd[:, :])
            nc.gpsimd.dma_start(out=bias_sb, in_=biasd[:, :])
            dup_wave_kt1(0)
            load_wave_inputs(1)
            dup_wave_kt1(1)

            mg_tiles = [None] * NG

            def emit_group_tail(g):
                mt = mt_ps.tile([128, 128], F16, name="mt")
                _flush(nc.tensor.transpose(mt, mg_tiles[g], ident_sb))
                ow = owpool.tile([128, 128], F16, name="ow")
                with nc.allow_low_precision(reason="fp16 output"):
                    nc.vector.tensor_scalar(
                        out=ow,
                        in0=mt,
                        scalar1=-1.0,
                        scalar2=bias_sb,
                        op0=mybir.AluOpType.mult,
                        op1=mybir.AluOpType.add,
                    )
                seg = out[:, g * GRP * TC : (g + 1) * GRP * TC]
                nc.sync.dma_start(
                    out=seg.rearrange("k (c tc) -> c k tc", c=GRP),
                    in_=ow,
                )

            for w in range(NW):
                xd3 = xds[w]
                xe3 = xes[w]
                if w + 2 < NW:
                    load_wave_inputs(w + 2)
                    dup_wave_kt1(w + 2)
                for tcl in range(WAVE):
                    c = w * WAVE + tcl
                    g = c // GRP
                    cl = c % GRP
                    off = tcl * TC
                    if tcl == 0:
                        pe_observe(xd3[0:2, 0, 0:2])
                        pe_observe(xd3[0:2, 1, 0:2])
                        if w == 0:
                            pe_observe(ident_sb[0:2, 0:2])
                    if tcl == 1:
                        pe_observe(xe3[0:2, 0, 0:2])

                    ps = conv_ps.tile([128, 1024], f32, name="ps", tag="ps")
                    for dr in range(2):
                        lhsT = xd3[:, :, off + 4 * dr : off + 4 * dr + TC]
                        for h in range(2):
                            pe_matmul(
                                ps[:, 512 * h : 512 * h + 512],
                                lhsT,
                                w_sb[:, :, dr * 1024 + h * 512 : dr * 1024 + h * 512 + 512],
                                start=(dr == 0),
                                stop=False,
                                perf_mode=DR,
                            )
                    lhsT3 = xe3[:, :, off : off + TC]
                    for h in range(2):
                        mm_i = pe_matmul(
                            ps[:, 512 * h : 512 * h + 512],
                            lhsT3,
                            wt_sb[:, :, 512 * h : 512 * h + 512],
                            start=False,
                            stop=True,
                            perf_mode=DR,
                        )
                    if tcl == WAVE - 2:
                        obs_after[0] = mm_i

                    sq = sqpool.tile([128, 1024], F16, name="sq", tag="sq")
                    nc.scalar.activation(
                        sq, ps[:, 0:1024],
                        mybir.ActivationFunctionType.Square,
                        scale=ACT_SCALE,
                    )
                    if cl == 0:
                        mg_tiles[g] = mgpool.tile([128, GRP * 16], F16, name="mg")
                    sq3 = sq.rearrange("p (g c) -> p g c", g=16)
                    t1 = t1pool.tile([128, 16, 32], F16, name="t1", tag="t1")
                    with nc.allow_low_precision(reason="fp16 mahalanobis"):
                        # d-halves pairwise add runs in DVE 2x mode; the
                        # remaining 32-wide segmented reduce runs 1x
                        nc.vector.tensor_tensor(
                            out=t1,
                            in0=sq3[:, :, 0:32],
                            in1=sq3[:, :, 32:64],
                            op=mybir.AluOpType.add,
                        )
                        nc.vector.tensor_reduce(
                            out=mg_tiles[g][:, 16 * cl : 16 * cl + 16],
                            in_=t1,
                            axis=mybir.AxisListType.X,
                            op=mybir.AluOpType.add,
                        )

                    # emit group go's transpose/affine/store LAG chunks after
                    # its last reduce so the in-order PE never head-of-line
                    # blocks on the DVE
                    if c >= (GRP - 1) + LAG and (c - (GRP - 1) - LAG) % GRP == 0:
                        emit_group_tail((c - (GRP - 1) - LAG) // GRP)

            first_left = (NG * GRP - 1 - (GRP - 1) - LAG) // GRP + 1
            for go in range(max(first_left, 0), NG):
                emit_group_tail(go)
    nc.compile()
    return nc


def _prep_host(W, b, Sigma):
    """Fold L^{-1} into conv weights; quantize and pack fp8 operands."""
    W64 = W.astype(np.float64)
    b64 = b.astype(np.float64)
    S64 = Sigma.astype(np.float64)
    L = np.linalg.cholesky(S64)
    Li = np.linalg.inv(L)
    logdet = 2.0 * np.sum(np.log(np.diagonal(L, axis1=1, axis2=2)), axis=1)
    W2 = np.einsum("kdc,kcij->kdij", Li, W64)   # [K, C(d), C(ci), 9]
    b2 = np.einsum("kdc,kc->kd", Li, b64)       # [K, C]

    W2s = (SW * W2).astype(np.float32)          # [kg, d, ci, j]
    w_main = np.zeros((128, 2, 2048), np.float32)
    for joff in range(2):
        for kt in range(2):
            for dr in range(2):
                j = 4 * dr + joff + 2 * kt
                # [ci, (h k8) d] = W2s[kg, d, ci, j]
                blk = np.transpose(W2s[:, :, :, j], (2, 0, 1)).reshape(C, 1024)
                w_main[64 * joff : 64 * joff + 64, kt,
                       1024 * dr : 1024 * dr + 1024] = blk
    w_tail = np.zeros((66, 2, 1024), np.float32)
    w_tail[0:C, 0, :] = np.transpose(W2s[:, :, :, 8], (2, 0, 1)).reshape(C, 1024)
    w_tail[C, 0, :] = (SW * b2).astype(np.float32).reshape(1024)

    const = C * np.log(2.0 * np.pi) + logdet    # [K]
    bias_np = np.tile((-0.5 * const).astype(np.float32), 8).reshape(128, 1)
    return w_main.astype(NP8), w_tail.astype(NP8), bias_np


def _prep_x(x):
    """Quantize x once, build per-core shift-duplicated fp8 operands."""
    xq = np.asarray(x, np.float32)[0].astype(NP8)            # [C, T]
    xpad = np.zeros((C, AR + T + 40), NP8)
    xpad[:, AR : AR + T] = xq
    xd_all, xe_all = [], []
    ones_row = np.ones((1, XEW), NP8)
    zero_row = np.zeros((1, XEW), NP8)
    for i in range(NCORES):
        xd = np.empty((NW * 128, XW), NP8)
        xe = np.empty((NW * 66, 2, XEW), NP8)
        for w in range(NW):
            base = i * TLOC + w * WCOLS
            for joff in range(2):
                xd[128 * w + 64 * joff : 128 * w + 64 * joff + 64, :] = \
                    xpad[:, base + joff : base + joff + XW]
            tail = xpad[:, base + 8 : base + 8 + XEW]
            for kt in range(2):
                xe[66 * w : 66 * w + 64, kt, :] = tail
                xe[66 * w + 64, kt, :] = ones_row
                xe[66 * w + 65, kt, :] = zero_row
        xd_all.append(xd)
        xe_all.append(xe)
    return xd_all, xe_all


def _run(x, W, b, Sigma, trace=False):
    if "nc" not in _CACHE:
        _CACHE["nc"] = _build_program()
    nc = _CACHE["nc"]
    w_main, w_tail, bias_np = _prep_host(
        np.asarray(W, np.float32), np.asarray(b, np.float32),
        np.asarray(Sigma, np.float32))
    xd_all, xe_all = _prep_x(np.asarray(x, np.float32))
    ident = np.eye(128, dtype=NP16)

    in_maps = []
    for i in range(NCORES):
        in_maps.append(
            {
                "xdram": xd_all[i],
                "xedram": xe_all[i],
                "wdram": w_main,
                "wtdram": w_tail,
                "identd": ident,
                "biasd": bias_np,
            }
        )
    res = run_bass_kernel_spmd(
        nc, in_maps, core_ids=list(range(NCORES)), trace=trace
    )
    outs = [res.results[i]["out"].astype(np.float32) for i in range(NCORES)]
    full = np.concatenate(outs, axis=1)[None]   # [1, K, T]
    return full, res


def kernel(x, W, b, Sigma):
    out, _ = _run(x, W, b, Sigma, trace=bool(int(os.environ.get("BASS_TRACE", "0"))))
    return out


# revision 6
# speedup vs baseline: 1.8542x; 1.0268x over previous
"""Trainium2 Bass kernel for nn_Autoregression (16-state AR whitening log-prob).

Math: log_prob[b,k,t] = -0.5*(C*log(2pi) + logdet(Sigma_k)
+ es_k(t)^T Sigma_k^{-1} es_k(t)) with es = causal_conv(x, W, b).  Fold
L^{-1} into the conv (W2 = L^{-1} W, b2 = L^{-1} b) so mahalanobis =
sum_d conv(x; W2, b2)^2.

Device layout (per core, T sharded 8 ways with an 8-sample causal halo):
conv runs in fp8e4 DoubleRow matmuls (256-deep contraction per
instruction, 2 k-tiles along the free axis).  Contraction of 64 ci x 9
taps + bias = 577 rows packs as 2 full DoubleRows from a shift-duplicated
x tile (taps 0-3 / 4-7 via kt-plane shifts of 2) plus one 66-row
DoubleRow (tap 8 + ones-bias row).  Per 128-t chunk: 6 matmuls into a
2-bank PSUM tile [128 t, 1024 (2 halves x 8 states x 64 ch)].  ACT
squares the whole tile in one op (scale folds the fp8 weight scale and
the -0.5) -> fp16 SBUF; DVE folds the d-dim in two steps (a 2x-mode
tensor_tensor add of d-halves, then a 1x segmented reduce) -> [128,16]
fp16.  Every 8 chunks a fp16 PE transpose flips [128 t, 128 (c,k)] ->
[128, 128 t]; DVE applies -1 and the per-state constant (2x mode);
fp16 k-major stores, host converts to f32.
"""

import math
import os

import numpy as np
import ml_dtypes

import concourse.bass as bass
import concourse.bacc as bacc_mod
import concourse.mybir as mybir
import concourse.tile as tile
from concourse.bass_utils import run_bass_kernel_spmd
from concourse.tile_rust import add_dep_helper

K = 16          # states
C = 64          # channels
T = 65536       # time
AR = 8          # ar order (kernel size AR+1)
NCORES = 8
TLOC = T // NCORES          # 8192 outputs per core
TC = 128                    # outputs per chunk (matmul M)
WAVE = 16                   # chunks per wave (input tile granularity)
WCOLS = TC * WAVE           # 2048 outputs per wave
NW = TLOC // WCOLS          # waves per core
GRP = 8                     # chunks per transpose/store group
LAG = 3                     # chunks between a group's last reduce and its transpose
XW = WCOLS + 32             # xd3 plane width (halo + shift-copy margin, 16B-aligned)
XEW = WCOLS                 # xe3 plane width
SW = 64.0                   # fp8 weight scale

F8 = mybir.dt.float8e4
F16 = mybir.dt.float16
NP8 = mybir.dt.np(F8)
NP16 = mybir.dt.np(F16)
DR = mybir.MatmulPerfMode.DoubleRow

_CACHE: dict = {}


def _build_program():
    nc = bacc_mod.Bacc()
    f32 = mybir.dt.float32

    # x, shift-duplicated: partition p=(joff,ci), plane kt -> time shift
    # joff + 2*kt; per wave 128 rows of [2, XW]
    xdram = nc.declare_dram_parameter("xdram", [NW * 128, XW], F8, isOutput=False)
    # tail operand: rows 0-63 = x shifted by 8 (tap 8), row 64 = ones
    # (bias row), row 65 = pad; kt1 = copy of kt0 (zero-weighted)
    xedram = nc.declare_dram_parameter("xedram", [NW * 66, 2, XEW], F8, isOutput=False)
    # conv weights, moving operand: [row=(joff,ci), kt, dr*1024 + h*512 + k8*64 + d]
    wdram = nc.declare_dram_parameter("wdram", [128, 2, 2048], F8, isOutput=False)
    # tail weights: [row, kt, h*512 + k8*64 + d]; row 64 = bias, kt1 = 0
    wtdram = nc.declare_dram_parameter("wtdram", [66, 2, 1024], F8, isOutput=False)
    identd = nc.declare_dram_parameter("identd", [128, 128], F16, isOutput=False)
    # -0.5 * (C*log(2pi) + logdet_k) per transposed partition p -> k = p%16
    biasd = nc.declare_dram_parameter("biasd", [128, 1], f32, isOutput=False)
    out = nc.declare_dram_parameter("out", [K, TLOC], F16, isOutput=True)

    NG = NW * WAVE // GRP       # groups per core
    ACT_SCALE = 1.0 / (SW * math.sqrt(2.0))

    with tile.TileContext(nc) as tc:
        with (
            tc.tile_pool(name="singles", bufs=1) as singles,
            # one slot per wave: input DMAs never wait (no slot WAR/WAW)
            tc.tile_pool(name="xpool", bufs=NW) as xpool,
            tc.tile_pool(name="sqpool", bufs=6) as sqpool,
            tc.tile_pool(name="t1pool", bufs=4) as t1pool,
            tc.tile_pool(name="mgpool", bufs=2) as mgpool,
            tc.tile_pool(name="owpool", bufs=2) as owpool,
            tc.tile_pool(name="conv_ps", bufs=3, space="PSUM") as conv_ps,
            tc.tile_pool(name="mt_ps", bufs=1, space="PSUM") as mt_ps,
        ):
            # Matmuls must never be the first PE instruction to observe more
            # than one producer semaphore (1-wait ISA slots; bacc's event-sem
            # legalization costs sequencer time).  pe_observe() emits a tiny
            # 2x2 "reader" matmul whose operands come from a single
            # producer's tile; ordering edges pin readers ahead of the next
            # real matmul.
            scratch = mt_ps.tile([2, 128], f32)
            scratch2 = singles.tile([2, 128], mybir.dt.bfloat16)
            nc.vector.memset(scratch2, 0.0)
            pending = []
            obs_after = [None]

            def pe_observe(col):
                i = nc.tensor.matmul(
                    scratch[0:2, 0:2], col, col, start=True, stop=True
                )
                if obs_after[0] is not None:
                    # not earlier than late in the previous wave, or the PE
                    # FIFO head-of-line blocks on a DMA that hasn't landed
                    add_dep_helper(i.ins, obs_after[0].ins, sync=False)
                pending.append(i)

            def _flush(i):
                while pending:
                    add_dep_helper(i.ins, pending.pop().ins, sync=False)
                return i

            def pe_matmul(*args, **kw):
                return _flush(nc.tensor.matmul(*args, **kw))

            # dep-free warmup matmuls: keep the PE busy through the initial
            # input DMAs so HAM un-throttles before real work
            for _ in range(35):
                nc.tensor.matmul(
                    scratch[0:2, 0:128],
                    scratch2[0:2, 0:2],
                    scratch2[0:2, 0:128],
                    start=True,
                    stop=True,
                )

            w_sb = singles.tile([128, 2, 2048], F8)
            wt_sb = singles.tile([66, 2, 1024], F8)
            ident_sb = singles.tile([128, 128], F16)
            bias_sb = singles.tile([128, 1], f32)

            xds, xes = [], []
            for w in range(NW):
                xds.append(xpool.tile([128, 2, XW], F8, name="xd3"))
                xes.append(xpool.tile([66, 2, XEW], F8, name="xe3"))

            # All input loads ride the gpsimd SWDGE ring: software DGE fans
            # descriptors across all 16 DMA engines (~7x the bandwidth of a
            # single HWDGE ring).  Only kt-plane 0 of the x operand comes
            # from DRAM; kt1 (= kt0 shifted 2 cols) is synthesized by a DVE
            # uint16 shift-copy.
            U16 = mybir.dt.uint16

            def load_wave_inputs(w):
                nc.gpsimd.dma_start(
                    out=xds[w][:, 0, :], in_=xdram[128 * w : 128 * w + 128, :]
                )
                nc.gpsimd.dma_start(
                    out=xes[w], in_=xedram[66 * w : 66 * w + 66, :, :]
                )

            def dup_wave_kt1(w):
                nc.vector.tensor_copy(
                    xds[w][:, 1, 0 : XW - 16].bitcast(U16),
                    xds[w][:, 0, 2 : XW - 14].bitcast(U16),
                )

            # need-order on the SWDGE queue: x window, then the four conv
            # weight blocks in first-use order, then the tail weights; xe0 and
            # the small constants ride the (otherwise idle) sync HWDGE ring in
            # parallel
            nc.gpsimd.dma_start(
                out=xds[0][:, 0, :], in_=xdram[0:128, :]
            )
            for blk in range(4):
                nc.gpsimd.dma_start(
                    out=w_sb[:, :, 512 * blk : 512 * blk + 512],
                    in_=wdram[:, :, 512 * blk : 512 * blk + 512],
                )
            nc.gpsimd.dma_start(out=wt_sb, in_=wtdram[:, :, :])
            nc.sync.dma_start(out=xes[0], in_=xedram[0:66, :, :])
            nc.sync.dma_start(out=ident_sb, in_=identd[:, :])
            nc.sync.dma_start(out=bias_sb, in_=biasd[:, :])
            dup_wave_kt1(0)
            load_wave_inputs(1)
            dup_wave_kt1(1)

            mg_tiles = [None] * NG

            def emit_group_tail(g):
                mt = mt_ps.tile([128, 128], F16, name="mt")
                _flush(nc.tensor.transpose(mt, mg_tiles[g], ident_sb))
                ow = owpool.tile([128, 128], F16, name="ow")
                with nc.allow_low_precision(reason="fp16 output"):
                    nc.vector.tensor_scalar(
                        out=ow,
                        in0=mt,
                        scalar1=-1.0,
                        scalar2=bias_sb,
                        op0=mybir.AluOpType.mult,
                        op1=mybir.AluOpType.add,
                    )
                seg = out[:, g * GRP * TC : (g + 1) * GRP * TC]
                nc.sync.dma_start(
                    out=seg.rearrange("k (c tc) -> c k tc", c=GRP),
                    in_=ow,
                )

            def emit_dr12(xd3, ps, off):
                for dr in range(2):
                    lhsT = xd3[:, :, off + 4 * dr : off + 4 * dr + TC]
                    for h in range(2):
                        pe_matmul(
                            ps[:, 512 * h : 512 * h + 512],
                            lhsT,
                            w_sb[:, :, dr * 1024 + h * 512 : dr * 1024 + h * 512 + 512],
                            start=(dr == 0),
                            stop=False,
                            perf_mode=DR,
                        )

            def emit_dr3(xe3, ps, off):
                for h in range(2):
                    mm_i = pe_matmul(
                        ps[:, 512 * h : 512 * h + 512],
                        xe3[:, :, off : off + TC],
                        wt_sb[:, :, 512 * h : 512 * h + 512],
                        start=False,
                        stop=True,
                        perf_mode=DR,
                    )
                return mm_i

            staged = []

            for w in range(NW):
                xd3 = xds[w]
                xe3 = xes[w]
                if w + 2 < NW:
                    load_wave_inputs(w + 2)
                    dup_wave_kt1(w + 2)
                for tcl in range(WAVE):
                    c = w * WAVE + tcl
                    g = c // GRP
                    cl = c % GRP
                    off = tcl * TC
                    if tcl == 0:
                        pe_observe(xd3[0:2, 0, 0:2])
                        pe_observe(xd3[0:2, 1, 0:2])
                        if w == 0:
                            pe_observe(ident_sb[0:2, 0:2])
                    if tcl == 1 and w > 0:
                        pe_observe(xe3[0:2, 0, 0:2])

                    def emit_elementwise(cc, ps_c):
                        gg, ccl = cc // GRP, cc % GRP
                        sq = sqpool.tile([128, 1024], F16, name="sq", tag="sq")
                        nc.scalar.activation(
                            sq, ps_c[:, 0:1024],
                            mybir.ActivationFunctionType.Square,
                            scale=ACT_SCALE,
                        )
                        if ccl == 0:
                            mg_tiles[gg] = mgpool.tile(
                                [128, GRP * 16], F16, name="mg")
                        sq3 = sq.rearrange("p (g c) -> p g c", g=16)
                        t1 = t1pool.tile([128, 16, 32], F16, name="t1", tag="t1")
                        with nc.allow_low_precision(reason="fp16 mahalanobis"):
                            # d-halves pairwise add runs in DVE 2x mode; the
                            # remaining 32-wide segmented reduce runs 1x
                            nc.vector.tensor_tensor(
                                out=t1,
                                in0=sq3[:, :, 0:32],
                                in1=sq3[:, :, 32:64],
                                op=mybir.AluOpType.add,
                            )
                            nc.vector.tensor_reduce(
                                out=mg_tiles[gg][:, 16 * ccl : 16 * ccl + 16],
                                in_=t1,
                                axis=mybir.AxisListType.X,
                                op=mybir.AluOpType.add,
                            )

                    ps = conv_ps.tile([128, 1024], f32, name="ps", tag="ps")
                    emit_dr12(xd3, ps, off)
                    if w == 0 and tcl < 3:
                        # wave-0 pipelined start: queue DR1/DR2 for chunks
                        # 0-2 back to back (they only need xd + w blocks);
                        # their DR3 tails run once wt/xe0 land
                        staged.append((c, ps, off))
                        if tcl < 2:
                            continue
                        pe_observe(xe3[0:2, 0, 0:2])
                        for c_s, ps_s, off_s in staged:
                            mm_i = emit_dr3(xe3, ps_s, off_s)
                            emit_elementwise(c_s, ps_s)
                    else:
                        mm_i = emit_dr3(xe3, ps, off)
                        emit_elementwise(c, ps)
                    if tcl == WAVE - 2:
                        obs_after[0] = mm_i

                    # emit group go's transpose/affine/store LAG chunks after
                    # its last reduce so the in-order PE never head-of-line
                    # blocks on the DVE
                    if c >= (GRP - 1) + LAG and (c - (GRP - 1) - LAG) % GRP == 0:
                        emit_group_tail((c - (GRP - 1) - LAG) // GRP)

            first_left = (NG * GRP - 1 - (GRP - 1) - LAG) // GRP + 1
            for go in range(max(first_left, 0), NG):
                emit_group_tail(go)
    nc.compile()
    return nc


def _prep_host(W, b, Sigma):
    """Fold L^{-1} into conv weights; quantize and pack fp8 operands."""
    W64 = W.astype(np.float64)
    b64 = b.astype(np.float64)
    S64 = Sigma.astype(np.float64)
    L = np.linalg.cholesky(S64)
    Li = np.linalg.inv(L)
    logdet = 2.0 * np.sum(np.log(np.diagonal(L, axis1=1, axis2=2)), axis=1)
    W2 = np.einsum("kdc,kcij->kdij", Li, W64)   # [K, C(d), C(ci), 9]
    b2 = np.einsum("kdc,kc->kd", Li, b64)       # [K, C]

    W2s = (SW * W2).astype(np.float32)          # [kg, d, ci, j]
    w_main = np.zeros((128, 2, 2048), np.float32)
    for joff in range(2):
        for kt in range(2):
            for dr in range(2):
                j = 4 * dr + joff + 2 * kt
                # [ci, (h k8) d] = W2s[kg, d, ci, j]
                blk = np.transpose(W2s[:, :, :, j], (2, 0, 1)).reshape(C, 1024)
                w_main[64 * joff : 64 * joff + 64, kt,
                       1024 * dr : 1024 * dr + 1024] = blk
    w_tail = np.zeros((66, 2, 1024), np.float32)
    w_tail[0:C, 0, :] = np.transpose(W2s[:, :, :, 8], (2, 0, 1)).reshape(C, 1024)
    w_tail[C, 0, :] = (SW * b2).astype(np.float32).reshape(1024)

    const = C * np.log(2.0 * np.pi) + logdet    # [K]
    bias_np = np.tile((-0.5 * const).astype(np.float32), 8).reshape(128, 1)
    return w_main.astype(NP8), w_tail.astype(NP8), bias_np


def _prep_x(x):
    """Quantize x once, build per-core shift-duplicated fp8 operands."""
    xq = np.asarray(x, np.float32)[0].astype(NP8)            # [C, T]
    xpad = np.zeros((C, AR + T + 40), NP8)
    xpad[:, AR : AR + T] = xq
    xd_all, xe_all = [], []
    ones_row = np.ones((1, XEW), NP8)
    zero_row = np.zeros((1, XEW), NP8)
    for i in range(NCORES):
        xd = np.empty((NW * 128, XW), NP8)
        xe = np.empty((NW * 66, 2, XEW), NP8)
        for w in range(NW):
            base = i * TLOC + w * WCOLS
            for joff in range(2):
                xd[128 * w + 64 * joff : 128 * w + 64 * joff + 64, :] = \
                    xpad[:, base + joff : base + joff + XW]
            tail = xpad[:, base + 8 : base + 8 + XEW]
            for kt in range(2):
                xe[66 * w : 66 * w + 64, kt, :] = tail
                xe[66 * w + 64, kt, :] = ones_row
                xe[66 * w + 65, kt, :] = zero_row
        xd_all.append(xd)
        xe_all.append(xe)
    return xd_all, xe_all


def _run(x, W, b, Sigma, trace=False):
    if "nc" not in _CACHE:
        _CACHE["nc"] = _build_program()
    nc = _CACHE["nc"]
    w_main, w_tail, bias_np = _prep_host(
        np.asarray(W, np.float32), np.asarray(b, np.float32),
        np.asarray(Sigma, np.float32))
    xd_all, xe_all = _prep_x(np.asarray(x, np.float32))
    ident = np.eye(128, dtype=NP16)

    in_maps = []
    for i in range(NCORES):
        in_maps.append(
            {
                "xdram": xd_all[i],
                "xedram": xe_all[i],
                "wdram": w_main,
                "wtdram": w_tail,
                "identd": ident,
                "biasd": bias_np,
            }
        )
    res = run_bass_kernel_spmd(
        nc, in_maps, core_ids=list(range(NCORES)), trace=trace
    )
    outs = [res.results[i]["out"].astype(np.float32) for i in range(NCORES)]
    full = np.concatenate(outs, axis=1)[None]   # [1, K, T]
    return full, res


def kernel(x, W, b, Sigma):
    out, _ = _run(x, W, b, Sigma, trace=bool(int(os.environ.get("BASS_TRACE", "0"))))
    return out


# revision 7
# speedup vs baseline: 1.8545x; 1.0002x over previous
"""Trainium2 Bass kernel for nn_Autoregression (16-state AR whitening log-prob).

Math: log_prob[b,k,t] = -0.5*(C*log(2pi) + logdet(Sigma_k)
+ es_k(t)^T Sigma_k^{-1} es_k(t)) with es = causal_conv(x, W, b).  Fold
L^{-1} into the conv (W2 = L^{-1} W, b2 = L^{-1} b) so mahalanobis =
sum_d conv(x; W2, b2)^2.

Device layout (per core, T sharded 8 ways with an 8-sample causal halo):
conv runs in fp8e4 DoubleRow matmuls (256-deep contraction per
instruction, 2 k-tiles along the free axis).  Contraction of 64 ci x 9
taps + bias = 577 rows packs as 2 full DoubleRows from a shift-duplicated
x tile (taps 0-3 / 4-7 via kt-plane shifts of 2) plus one 66-row
DoubleRow (tap 8 + ones-bias row).  Per 128-t chunk: 6 matmuls into a
2-bank PSUM tile [128 t, 1024 (2 halves x 8 states x 64 ch)].  ACT
squares the whole tile in one op (scale folds the fp8 weight scale and
the -0.5) -> fp16 SBUF; DVE folds the d-dim in two steps (a 2x-mode
tensor_tensor add of d-halves, then a 1x segmented reduce) -> [128,16]
fp16.  Every 8 chunks a fp16 PE transpose flips [128 t, 128 (c,k)] ->
[128, 128 t]; DVE applies -1 and the per-state constant (2x mode);
fp16 k-major stores, host converts to f32.
"""

import math
import os

import numpy as np
import ml_dtypes

import concourse.bass as bass
import concourse.bacc as bacc_mod
import concourse.mybir as mybir
import concourse.tile as tile
from concourse.bass_utils import run_bass_kernel_spmd
from concourse.tile_rust import add_dep_helper

K = 16          # states
C = 64          # channels
T = 65536       # time
AR = 8          # ar order (kernel size AR+1)
NCORES = 8
TLOC = T // NCORES          # 8192 outputs per core
TC = 128                    # outputs per chunk (matmul M)
WAVE = 16                   # chunks per wave (input tile granularity)
WCOLS = TC * WAVE           # 2048 outputs per wave
NW = TLOC // WCOLS          # waves per core
GRP = 8                     # chunks per transpose/store group
LAG = 3                     # chunks between a group's last reduce and its transpose
XW = WCOLS + 32             # xd3 plane width (halo + shift-copy margin, 16B-aligned)
XEW = WCOLS                 # xe3 plane width
SW = 64.0                   # fp8 weight scale

F8 = mybir.dt.float8e4
F16 = mybir.dt.float16
NP8 = mybir.dt.np(F8)
NP16 = mybir.dt.np(F16)
DR = mybir.MatmulPerfMode.DoubleRow

_CACHE: dict = {}


def _build_program():
    nc = bacc_mod.Bacc()
    f32 = mybir.dt.float32

    # x, shift-duplicated: partition p=(joff,ci), plane kt -> time shift
    # joff + 2*kt; per wave 128 rows of [2, XW]
    xdram = nc.declare_dram_parameter("xdram", [NW * 128, XW], F8, isOutput=False)
    # tail operand: rows 0-63 = x shifted by 8 (tap 8), row 64 = ones
    # (bias row), row 65 = pad; kt1 = copy of kt0 (zero-weighted)
    xedram = nc.declare_dram_parameter("xedram", [NW * 66, 2, XEW], F8, isOutput=False)
    # conv weights, moving operand: [row=(joff,ci), kt, dr*1024 + h*512 + k8*64 + d]
    wdram = nc.declare_dram_parameter("wdram", [128, 2, 2048], F8, isOutput=False)
    # tail weights: [row, kt, h*512 + k8*64 + d]; row 64 = bias, kt1 = 0
    wtdram = nc.declare_dram_parameter("wtdram", [66, 2, 1024], F8, isOutput=False)
    identd = nc.declare_dram_parameter("identd", [128, 128], F16, isOutput=False)
    # -0.5 * (C*log(2pi) + logdet_k) per transposed partition p -> k = p%16
    biasd = nc.declare_dram_parameter("biasd", [128, 1], f32, isOutput=False)
    out = nc.declare_dram_parameter("out", [K, TLOC], F16, isOutput=True)

    NG = NW * WAVE // GRP       # groups per core
    ACT_SCALE = 1.0 / (SW * math.sqrt(2.0))

    with tile.TileContext(nc) as tc:
        with (
            tc.tile_pool(name="singles", bufs=1) as singles,
            # one slot per wave: input DMAs never wait (no slot WAR/WAW)
            tc.tile_pool(name="xpool", bufs=NW) as xpool,
            tc.tile_pool(name="sqpool", bufs=6) as sqpool,
            tc.tile_pool(name="t1pool", bufs=4) as t1pool,
            tc.tile_pool(name="mgpool", bufs=2) as mgpool,
            tc.tile_pool(name="owpool", bufs=2) as owpool,
            tc.tile_pool(name="conv_ps", bufs=3, space="PSUM") as conv_ps,
            tc.tile_pool(name="mt_ps", bufs=1, space="PSUM") as mt_ps,
        ):
            # Matmuls must never be the first PE instruction to observe more
            # than one producer semaphore (1-wait ISA slots; bacc's event-sem
            # legalization costs sequencer time).  pe_observe() emits a tiny
            # 2x2 "reader" matmul whose operands come from a single
            # producer's tile; ordering edges pin readers ahead of the next
            # real matmul.
            scratch = mt_ps.tile([2, 128], f32)
            scratch2 = singles.tile([2, 128], mybir.dt.bfloat16)
            nc.vector.memset(scratch2, 0.0)
            pending = []
            obs_after = [None]

            def pe_observe(col):
                i = nc.tensor.matmul(
                    scratch[0:2, 0:2], col, col, start=True, stop=True
                )
                if obs_after[0] is not None:
                    # not earlier than late in the previous wave, or the PE
                    # FIFO head-of-line blocks on a DMA that hasn't landed
                    add_dep_helper(i.ins, obs_after[0].ins, sync=False)
                pending.append(i)

            def _flush(i):
                while pending:
                    add_dep_helper(i.ins, pending.pop().ins, sync=False)
                return i

            def pe_matmul(*args, **kw):
                return _flush(nc.tensor.matmul(*args, **kw))

            # dep-free warmup matmuls: keep the PE busy through the initial
            # input DMAs so HAM un-throttles before real work
            for _ in range(35):
                nc.tensor.matmul(
                    scratch[0:2, 0:128],
                    scratch2[0:2, 0:2],
                    scratch2[0:2, 0:128],
                    start=True,
                    stop=True,
                )

            w_sb = singles.tile([128, 2, 2048], F8)
            wt_sb = singles.tile([66, 2, 1024], F8)
            ident_sb = singles.tile([128, 128], F16)
            bias_sb = singles.tile([128, 1], f32)

            xds, xes = [], []
            for w in range(NW):
                xds.append(xpool.tile([128, 2, XW], F8, name="xd3"))
                xes.append(xpool.tile([66, 2, XEW], F8, name="xe3"))

            # All input loads ride the gpsimd SWDGE ring: software DGE fans
            # descriptors across all 16 DMA engines (~7x the bandwidth of a
            # single HWDGE ring).  Only kt-plane 0 of the x operand comes
            # from DRAM; kt1 (= kt0 shifted 2 cols) is synthesized by a DVE
            # uint16 shift-copy.
            U16 = mybir.dt.uint16

            def load_wave_inputs(w):
                nc.gpsimd.dma_start(
                    out=xds[w][:, 0, :], in_=xdram[128 * w : 128 * w + 128, :]
                )
                nc.gpsimd.dma_start(
                    out=xes[w], in_=xedram[66 * w : 66 * w + 66, :, :]
                )

            def dup_wave_kt1(w):
                nc.vector.tensor_copy(
                    xds[w][:, 1, 0 : XW - 16].bitcast(U16),
                    xds[w][:, 0, 2 : XW - 14].bitcast(U16),
                )

            # need-order on the SWDGE queue: x window, then the four conv
            # weight blocks in first-use order, then the tail weights; xe0 and
            # the small constants ride the (otherwise idle) sync HWDGE ring in
            # parallel
            nc.gpsimd.dma_start(
                out=xds[0][:, 0, :], in_=xdram[0:128, :]
            )
            for blk in range(4):
                nc.gpsimd.dma_start(
                    out=w_sb[:, :, 512 * blk : 512 * blk + 512],
                    in_=wdram[:, :, 512 * blk : 512 * blk + 512],
                )
            nc.gpsimd.dma_start(out=wt_sb, in_=wtdram[:, :, :])
            nc.sync.dma_start(out=ident_sb, in_=identd[:, :])
            nc.sync.dma_start(out=bias_sb, in_=biasd[:, :])
            nc.sync.dma_start(out=xes[0], in_=xedram[0:66, :, :])
            dup_wave_kt1(0)
            load_wave_inputs(1)
            dup_wave_kt1(1)

            mg_tiles = [None] * NG

            def emit_group_tail(g):
                mt = mt_ps.tile([128, 128], F16, name="mt")
                _flush(nc.tensor.transpose(mt, mg_tiles[g], ident_sb))
                ow = owpool.tile([128, 128], F16, name="ow")
                with nc.allow_low_precision(reason="fp16 output"):
                    nc.vector.tensor_scalar(
                        out=ow,
                        in0=mt,
                        scalar1=-1.0,
                        scalar2=bias_sb,
                        op0=mybir.AluOpType.mult,
                        op1=mybir.AluOpType.add,
                    )
                seg = out[:, g * GRP * TC : (g + 1) * GRP * TC]
                nc.sync.dma_start(
                    out=seg.rearrange("k (c tc) -> c k tc", c=GRP),
                    in_=ow,
                )

            def emit_dr12(xd3, ps, off):
                for dr in range(2):
                    lhsT = xd3[:, :, off + 4 * dr : off + 4 * dr + TC]
                    for h in range(2):
                        pe_matmul(
                            ps[:, 512 * h : 512 * h + 512],
                            lhsT,
                            w_sb[:, :, dr * 1024 + h * 512 : dr * 1024 + h * 512 + 512],
                            start=(dr == 0),
                            stop=False,
                            perf_mode=DR,
                        )

            def emit_dr3(xe3, ps, off):
                for h in range(2):
                    mm_i = pe_matmul(
                        ps[:, 512 * h : 512 * h + 512],
                        xe3[:, :, off : off + TC],
                        wt_sb[:, :, 512 * h : 512 * h + 512],
                        start=False,
                        stop=True,
                        perf_mode=DR,
                    )
                return mm_i

            staged = []

            for w in range(NW):
                xd3 = xds[w]
                xe3 = xes[w]
                if w + 2 < NW:
                    load_wave_inputs(w + 2)
                    dup_wave_kt1(w + 2)
                for tcl in range(WAVE):
                    c = w * WAVE + tcl
                    g = c // GRP
                    cl = c % GRP
                    off = tcl * TC
                    if tcl == 0:
                        pe_observe(xd3[0:2, 0, 0:2])
                        pe_observe(xd3[0:2, 1, 0:2])
                    if tcl == 1 and w > 0:
                        pe_observe(xe3[0:2, 0, 0:2])
                    if w == 0 and tcl == 4:
                        # ident is only needed by the first transpose (~chunk
                        # 10); observing it early would gate chunk 0 on its DMA
                        pe_observe(ident_sb[0:2, 0:2])

                    def emit_elementwise(cc, ps_c):
                        gg, ccl = cc // GRP, cc % GRP
                        sq = sqpool.tile([128, 1024], F16, name="sq", tag="sq")
                        nc.scalar.activation(
                            sq, ps_c[:, 0:1024],
                            mybir.ActivationFunctionType.Square,
                            scale=ACT_SCALE,
                        )
                        if ccl == 0:
                            mg_tiles[gg] = mgpool.tile(
                                [128, GRP * 16], F16, name="mg")
                        sq3 = sq.rearrange("p (g c) -> p g c", g=16)
                        t1 = t1pool.tile([128, 16, 32], F16, name="t1", tag="t1")
                        with nc.allow_low_precision(reason="fp16 mahalanobis"):
                            # d-halves pairwise add runs in DVE 2x mode; the
                            # remaining 32-wide segmented reduce runs 1x
                            nc.vector.tensor_tensor(
                                out=t1,
                                in0=sq3[:, :, 0:32],
                                in1=sq3[:, :, 32:64],
                                op=mybir.AluOpType.add,
                            )
                            nc.vector.tensor_reduce(
                                out=mg_tiles[gg][:, 16 * ccl : 16 * ccl + 16],
                                in_=t1,
                                axis=mybir.AxisListType.X,
                                op=mybir.AluOpType.add,
                            )

                    ps = conv_ps.tile([128, 1024], f32, name="ps", tag="ps")
                    emit_dr12(xd3, ps, off)
                    if w == 0 and tcl < 3:
                        # wave-0 pipelined start: queue DR1/DR2 for chunks
                        # 0-2 back to back (they only need xd + w blocks);
                        # their DR3 tails run once wt/xe0 land
                        staged.append((c, ps, off))
                        if tcl < 2:
                            continue
                        pe_observe(xe3[0:2, 0, 0:2])
                        for c_s, ps_s, off_s in staged:
                            mm_i = emit_dr3(xe3, ps_s, off_s)
                            emit_elementwise(c_s, ps_s)
                    else:
                        mm_i = emit_dr3(xe3, ps, off)
                        emit_elementwise(c, ps)
                    if tcl == WAVE - 2:
                        obs_after[0] = mm_i

                    # emit group go's transpose/affine/store LAG chunks after
                    # its last reduce so the in-order PE never head-of-line
                    # blocks on the DVE
                    if c >= (GRP - 1) + LAG and (c - (GRP - 1) - LAG) % GRP == 0:
                        emit_group_tail((c - (GRP - 1) - LAG) // GRP)

            first_left = (NG * GRP - 1 - (GRP - 1) - LAG) // GRP + 1
            for go in range(max(first_left, 0), NG):
                emit_group_tail(go)
    nc.compile()
    return nc


def _prep_host(W, b, Sigma):
    """Fold L^{-1} into conv weights; quantize and pack fp8 operands."""
    W64 = W.astype(np.float64)
    b64 = b.astype(np.float64)
    S64 = Sigma.astype(np.float64)
    L = np.linalg.cholesky(S64)
    Li = np.linalg.inv(L)
    logdet = 2.0 * np.sum(np.log(np.diagonal(L, axis1=1, axis2=2)), axis=1)
    W2 = np.einsum("kdc,kcij->kdij", Li, W64)   # [K, C(d), C(ci), 9]
    b2 = np.einsum("kdc,kc->kd", Li, b64)       # [K, C]

    W2s = (SW * W2).astype(np.float32)          # [kg, d, ci, j]
    w_main = np.zeros((128, 2, 2048), np.float32)
    for joff in range(2):
        for kt in range(2):
            for dr in range(2):
                j = 4 * dr + joff + 2 * kt
                # [ci, (h k8) d] = W2s[kg, d, ci, j]
                blk = np.transpose(W2s[:, :, :, j], (2, 0, 1)).reshape(C, 1024)
                w_main[64 * joff : 64 * joff + 64, kt,
                       1024 * dr : 1024 * dr + 1024] = blk
    w_tail = np.zeros((66, 2, 1024), np.float32)
    w_tail[0:C, 0, :] = np.transpose(W2s[:, :, :, 8], (2, 0, 1)).reshape(C, 1024)
    w_tail[C, 0, :] = (SW * b2).astype(np.float32).reshape(1024)

    const = C * np.log(2.0 * np.pi) + logdet    # [K]
    bias_np = np.tile((-0.5 * const).astype(np.float32), 8).reshape(128, 1)
    return w_main.astype(NP8), w_tail.astype(NP8), bias_np


def _prep_x(x):
    """Quantize x once, build per-core shift-duplicated fp8 operands."""
    xq = np.asarray(x, np.float32)[0].astype(NP8)            # [C, T]
    xpad = np.zeros((C, AR + T + 40), NP8)
    xpad[:, AR : AR + T] = xq
    xd_all, xe_all = [], []
    ones_row = np.ones((1, XEW), NP8)
    zero_row = np.zeros((1, XEW), NP8)
    for i in range(NCORES):
        xd = np.empty((NW * 128, XW), NP8)
        xe = np.empty((NW * 66, 2, XEW), NP8)
        for w in range(NW):
            base = i * TLOC + w * WCOLS
            for joff in range(2):
                xd[128 * w + 64 * joff : 128 * w + 64 * joff + 64, :] = \
                    xpad[:, base + joff : base + joff + XW]
            tail = xpad[:, base + 8 : base + 8 + XEW]
            for kt in range(2):
                xe[66 * w : 66 * w + 64, kt, :] = tail
                xe[66 * w + 64, kt, :] = ones_row
                xe[66 * w + 65, kt, :] = zero_row
        xd_all.append(xd)
        xe_all.append(xe)
    return xd_all, xe_all


def _run(x, W, b, Sigma, trace=False):
    if "nc" not in _CACHE:
        _CACHE["nc"] = _build_program()
    nc = _CACHE["nc"]
    w_main, w_tail, bias_np = _prep_host(
        np.asarray(W, np.float32), np.asarray(b, np.float32),
        np.asarray(Sigma, np.float32))
    xd_all, xe_all = _prep_x(np.asarray(x, np.float32))
    ident = np.eye(128, dtype=NP16)

    in_maps = []
    for i in range(NCORES):
        in_maps.append(
            {
                "xdram": xd_all[i],
                "xedram": xe_all[i],
                "wdram": w_main,
                "wtdram": w_tail,
                "identd": ident,
                "biasd": bias_np,
            }
        )
    res = run_bass_kernel_spmd(
        nc, in_maps, core_ids=list(range(NCORES)), trace=trace
    )
    outs = [res.results[i]["out"].astype(np.float32) for i in range(NCORES)]
    full = np.concatenate(outs, axis=1)[None]   # [1, K, T]
    return full, res


def kernel(x, W, b, Sigma):
    out, _ = _run(x, W, b, Sigma, trace=bool(int(os.environ.get("BASS_TRACE", "0"))))
    return out
